# revision 1
# baseline (speedup 1.0000x reference)
"""AttentionBlock (GroupNorm + single-head attention + proj + residual) on 8 TRN2
NeuronCores.

Reference computation (B=16, C=512, H=W=32, N=H*W=1024, 32 groups):
    h   = group_norm(x, gamma, beta)                      # [B,C,H,W]
    qkv = conv1x1(h, w_qkv) + b_qkv                       # [B,3C,H,W]
    s   = q^T k / sqrt(C); a = softmax(s, axis=-1)        # [B,N,N]
    o   = v @ a^T; out = x + conv1x1(o, w_proj) + b_proj  # [B,C,H,W]

Sharding: pure data-parallel over batch. B=16 -> 2 batch elements per core,
weights replicated, no collectives. Each core runs the full block on its
2 batch elements; host scatters x and gathers out.

Device layout (per batch element, all [partition, free]):
    x, h      : [c, n]  as 4 tiles of [128, 1024]
    q, k      : [c, n]  (c here = head dim) 4 x [128, 1024]
    vT        : [n, c]  8 x [128, 512]   (computed directly via swapped matmul)
    sT=exp(.) : [j, i]  8 x [128, 1024]  (scores transposed: softmax dim on partitions)
    denom     : ones-matmul over j  -> [128(bcast), 1024] -> reciprocal
    av        : [c, i]  4 x [128, 1024] = vT^T @ eT, scaled by recip
    out       : x + w_projT^T @ av + b_eff
This layout chain is transpose-free; softmax normalization is applied after the
AV matmul (division commutes with the linear map), and the v/proj biases fold
into a single host-computed per-channel bias b_eff = w_proj @ b_v + b_proj.

Compute dtype bf16 (inputs/weights rounded to bf16, f32 PSUM accumulation);
GroupNorm statistics are computed in f32.

Emission order interleaves batch 1's GroupNorm into batch 0's attention phases
so the (in-order) engine streams keep TensorE fed across the batch boundary.
PSUM evacuations are split across DVE (q, av, proj+residual) and ACT
(k, vT via Identity-with-bias, exp) so they drain in parallel.
"""

import sys

for _p in ("/opt/trn_rl_repo", "/opt/pypackages"):
    if _p not in sys.path:
        sys.path.append(_p)

import numpy as np
import ml_dtypes

import concourse.bass as bass
import concourse.bacc as bacc
import concourse.tile as tile
from concourse import mybir

AF = mybir.ActivationFunctionType
OP = mybir.AluOpType
F32 = mybir.dt.float32
BF16 = mybir.dt.bfloat16
FP8 = mybir.dt.float8e4
LN16 = 2.772588722239781  # eT is stored as exp(s)/16 in fp8e4 to dodge the
                          # 448 saturation point; the softmax ratio is unchanged

N_CORES = 8
B, C, H, W = 16, 512, 32, 32
N = H * W               # 1024 pixels
BPC = B // N_CORES      # batch elements per core = 2
GROUPS = 32
EPS = 1e-5
KT = C // 128           # 4 contraction chunks over channels
NT = N // 128           # 8 chunks over pixels
SCALE = 1.0 / np.sqrt(np.float32(C))
QKV_FP8 = True    # qkv projection matmuls in fp8 DoubleRow (weights scaled x8)
PROJ_FP8 = True   # proj matmul in fp8 DoubleRow (weights scaled x8)
WS = 8.0          # fp8 weight pre-scale (keeps N(0,1/512) weights out of subnormals)


def build_nc():
    nc = bacc.Bacc("TRN2", target_bir_lowering=False)

    x_ext = nc.declare_dram_parameter("x", [BPC, C, N], F32, isOutput=False)
    wqkvT_ext = nc.declare_dram_parameter("wqkvT", [C, 3 * C], FP8 if QKV_FP8 else BF16, isOutput=False)
    wprojT_ext = nc.declare_dram_parameter("wprojT", [C, C], FP8 if PROJ_FP8 else BF16, isOutput=False)
    # consts: [128, 20] f32 = gamma | beta | b_q | b_k | b_eff, each [128, 4]
    consts_ext = nc.declare_dram_parameter("consts", [128, 20], F32, isOutput=False)
    # gmat: 16x16 block-diagonal of 1/16 (group-mean matrix); ones: all-ones
    gmat_ext = nc.declare_dram_parameter("gmat", [128, 128], BF16, isOutput=False)
    ones_ext = nc.declare_dram_parameter("ones", [128, 256], FP8, isOutput=False)
    out_ext = nc.declare_dram_parameter("out", [BPC, C, N], F32, isOutput=True)

    with tile.TileContext(nc) as tc:
        with (
            tc.tile_pool(name="wpool", bufs=1) as wpool,
            tc.tile_pool(name="xpool", bufs=2) as xpool,
            tc.tile_pool(name="hpool", bufs=2) as hpool,
            tc.tile_pool(name="qkpool", bufs=1) as qkpool,
            tc.tile_pool(name="vepool", bufs=1) as vepool,
            tc.tile_pool(name="avpool", bufs=1) as avpool,
            tc.tile_pool(name="opool", bufs=2) as opool,
            tc.tile_pool(name="stpool", bufs=2) as stpool,
            tc.tile_pool(name="ps_big", bufs=3, space="PSUM") as ps_big,
            tc.tile_pool(name="ps_gn", bufs=1, space="PSUM") as ps_gn,
        ):
            # small constants first (gmat gates the first matmul)
            consts = wpool.tile([128, 20], F32)
            nc.scalar.dma_start(out=consts, in_=consts_ext[:])
            gmat = wpool.tile([128, 128], BF16)
            nc.scalar.dma_start(out=gmat, in_=gmat_ext[:])
            ones = wpool.tile([128, 256], FP8)
            nc.scalar.dma_start(out=ones, in_=ones_ext[:])
            eps_sb = wpool.tile([128, 1], F32)
            nc.vector.memset(eps_sb, EPS)
            nln16_sb = wpool.tile([128, 1], F32)
            nc.vector.memset(nln16_sb, -LN16)
            gamma_sb = consts[:, 0:4]
            beta_sb = consts[:, 4:8]

            # x[0] chunks -> qkv weights -> x[1] chunks -> proj weights, all on
            # the sync queue in the order each is first needed.
            x_sbs = []
            for b in range(BPC):
                x_sb = xpool.tile([128, KT, N], F32, name="x_sb")
                x_sbs.append(x_sb)
            xr = [x_ext[b].rearrange("(ko p) n -> p ko n", p=128) for b in range(BPC)]
            for b in range(BPC):
                for ki in range(KT):
                    eng = nc.sync if ki % 2 == 0 else nc.gpsimd
                    eng.dma_start(out=x_sbs[b][:, ki, :], in_=xr[b][:, ki, :])

            wqkvT = wpool.tile([128, KT, 3 * C], FP8 if QKV_FP8 else BF16)
            nc.scalar.dma_start(out=wqkvT, in_=wqkvT_ext[:].rearrange("(ko p) f -> p ko f", p=128))
            wprojT = wpool.tile([128, KT, C], FP8 if PROJ_FP8 else BF16)
            nc.scalar.dma_start(out=wprojT, in_=wprojT_ext[:].rearrange("(ko p) f -> p ko f", p=128))

            h_sbs = [None, None]

            def emit_gn(b):
                # GroupNorm: per-partition mean/var via bn_stats, group-reduce
                # the 16-partition blocks with one small matmul against gmat,
                # then h = x*s + t in bf16.
                x_sb = x_sbs[b]
                mv = stpool.tile([128, KT, 2], F32, name="mv")
                for ki in range(KT):
                    stats = stpool.tile([128, 2, 6], F32, name="stats")
                    nc.vector.bn_stats(out=stats[:, 0, :], in_=x_sb[:, ki, 0:512])
                    nc.vector.bn_stats(out=stats[:, 1, :], in_=x_sb[:, ki, 512:1024])
                    nc.vector.bn_aggr(out=mv[:, ki, :], in_=stats)
                msq = stpool.tile([128, KT], F32, name="msq")
                nc.vector.tensor_tensor(msq, mv[:, :, 0], mv[:, :, 0], OP.mult)
                nc.vector.tensor_tensor(mv[:, :, 1], mv[:, :, 1], msq, OP.add)
                mv_bf = stpool.tile([128, KT * 2], BF16, name="mv_bf")
                nc.vector.tensor_copy(out=mv_bf, in_=mv.rearrange("p a b -> p (a b)"))
                gstat = ps_gn.tile([128, 128], F32, name="gstat", tag="gnps", bufs=1)[:, : KT * 2]
                nc.tensor.matmul(gstat, lhsT=gmat, rhs=mv_bf, start=True, stop=True)
                gs_sb = stpool.tile([128, KT * 2], F32, name="gs_sb")
                nc.vector.tensor_copy(out=gs_sb, in_=gstat)
                gmean = gs_sb[:, 0 : 2 * KT : 2]
                gex2 = gs_sb[:, 1 : 2 * KT : 2]
                gmsq = stpool.tile([128, KT], F32, name="gmsq")
                nc.vector.tensor_tensor(gmsq, gmean, gmean, OP.mult)
                gvar = stpool.tile([128, KT], F32, name="gvar")
                nc.vector.tensor_tensor(gvar, gex2, gmsq, OP.subtract)
                gstd = stpool.tile([128, KT], F32, name="gstd")
                nc.scalar.activation(out=gstd, in_=gvar, func=AF.Sqrt, bias=eps_sb)
                rstd = stpool.tile([128, KT], F32, name="rstd")
                nc.vector.reciprocal(out=rstd, in_=gstd)
                scl = stpool.tile([128, KT], F32, name="scl")
                nc.vector.tensor_tensor(scl, rstd, gamma_sb, OP.mult)
                mscl = stpool.tile([128, KT], F32, name="mscl")
                nc.vector.tensor_tensor(mscl, gmean, scl, OP.mult)
                sft = stpool.tile([128, KT], F32, name="sft")
                nc.vector.tensor_tensor(sft, beta_sb, mscl, OP.subtract)
                h_sb = hpool.tile([128, KT, N], FP8 if QKV_FP8 else BF16, name="h_sb")
                for ki in range(KT):
                    if ki < 2:
                        nc.vector.tensor_scalar(
                            out=h_sb[:, ki, :], in0=x_sb[:, ki, :],
                            scalar1=scl[:, ki : ki + 1], scalar2=sft[:, ki : ki + 1],
                            op0=OP.mult, op1=OP.add,
                        )
                    else:
                        nc.scalar.activation(
                            out=h_sb[:, ki, :], in_=x_sb[:, ki, :], func=AF.Identity,
                            bias=sft[:, ki : ki + 1], scale=scl[:, ki : ki + 1],
                        )
                h_sbs[b] = h_sb

            def emit_qk(b, qk):
                # q,k = wT.T @ h (+bias, bf16); q drains on DVE, k on ACT
                h_sb = h_sbs[b]
                q_sb, k_sb = qk
                for t, dst, bcol in ((1, k_sb, 12), (0, q_sb, 8)):
                    for oi in range(KT):
                        ps = ps_big.tile([128, N], F32, name="mmps")
                        w_sl = wqkvT[:, :, t * C + oi * 128 : t * C + (oi + 1) * 128]
                        if QKV_FP8:
                            for kk in range(2):
                                for ni in range(2):
                                    nc.tensor.matmul(
                                        ps[:, ni * 512 : (ni + 1) * 512],
                                        lhsT=w_sl[:, 2 * kk : 2 * kk + 2, :],
                                        rhs=h_sb[:, 2 * kk : 2 * kk + 2, ni * 512 : (ni + 1) * 512],
                                        start=(kk == 0), stop=(kk == 1),
                                        perf_mode=mybir.MatmulPerfMode.DoubleRow,
                                    )
                        else:
                            for ki in range(KT):
                                for ni in range(2):
                                    nc.tensor.matmul(
                                        ps[:, ni * 512 : (ni + 1) * 512],
                                        lhsT=w_sl[:, ki, :],
                                        rhs=h_sb[:, ki, ni * 512 : (ni + 1) * 512],
                                        start=(ki == 0), stop=(ki == KT - 1),
                                    )
                        for hf in range(2):
                            sl = slice(hf * 512, (hf + 1) * 512)
                            if t == 0:
                                nc.vector.tensor_scalar_add(
                                    out=dst[:, oi, sl], in0=ps[:, sl],
                                    scalar1=consts[:, bcol + oi : bcol + oi + 1],
                                )
                            else:
                                nc.scalar.activation(
                                    out=dst[:, oi, sl], in_=ps[:, sl], func=AF.Identity,
                                    bias=consts[:, bcol + oi : bcol + oi + 1],
                                )

            def emit_vt(b, vT_sb):
                # vT = h.T @ wvT (bf16), ACT Identity drain
                h_sb = h_sbs[b]
                for nn in range(NT // 2):
                    ps = ps_big.tile([128, N], F32, name="mmps")
                    for sub in range(2):
                        ni = 2 * nn + sub
                        if QKV_FP8:
                            for kk in range(2):
                                nc.tensor.matmul(
                                    ps[:, sub * 512 : (sub + 1) * 512],
                                    lhsT=h_sb[:, 2 * kk : 2 * kk + 2, ni * 128 : (ni + 1) * 128],
                                    rhs=wqkvT[:, 2 * kk : 2 * kk + 2, 2 * C : 3 * C],
                                    start=(kk == 0), stop=(kk == 1),
                                    perf_mode=mybir.MatmulPerfMode.DoubleRow,
                                )
                        else:
                            for ki in range(KT):
                                nc.tensor.matmul(
                                    ps[:, sub * 512 : (sub + 1) * 512],
                                    lhsT=h_sb[:, ki, ni * 128 : (ni + 1) * 128],
                                    rhs=wqkvT[:, ki, 2 * C : 3 * C],
                                    start=(ki == 0), stop=(ki == KT - 1),
                                )
                    nc.scalar.activation(
                        out=vT_sb[:, 2 * nn : 2 * nn + 2, :].rearrange("p a b -> p (a b)"),
                        in_=ps, func=AF.Identity,
                    )

            def emit_attn(b, qk, vT_sb):
                q_sb, k_sb = qk
                # eT = exp(k.T @ q * SCALE)  [j, i]
                eT_sb = vepool.tile([128, NT, N], FP8, name="eT_sb")
                # scores via fp8 DoubleRow (contracts 256 channels per matmul);
                # exp(s*SCALE - ln16) drains on ACT; the denominator matmuls for
                # tile ji-1 interleave behind the score matmuls of tile ji.
                ps_d = ps_big.tile([128, N], F32, name="psden", tag="mmps")

                def denom_mm(jj):
                    for ni in range(2):
                        nc.tensor.matmul(
                            ps_d[:, ni * 512 : (ni + 1) * 512],
                            lhsT=ones.rearrange("p (two f) -> p two f", two=2),
                            rhs=eT_sb[:, 2 * jj : 2 * jj + 2, ni * 512 : (ni + 1) * 512],
                            start=(jj == 0), stop=(jj == NT // 2 - 1),
                            perf_mode=mybir.MatmulPerfMode.DoubleRow,
                        )

                for ji in range(NT):
                    ps = ps_big.tile([128, N], F32, name="mmps")
                    for kk in range(2):
                        for ni in range(2):
                            nc.tensor.matmul(
                                ps[:, ni * 512 : (ni + 1) * 512],
                                lhsT=k_sb[:, 2 * kk : 2 * kk + 2, ji * 128 : (ji + 1) * 128],
                                rhs=q_sb[:, 2 * kk : 2 * kk + 2, ni * 512 : (ni + 1) * 512],
                                start=(kk == 0), stop=(kk == 1),
                                perf_mode=mybir.MatmulPerfMode.DoubleRow,
                            )
                    escale = SCALE / (WS * WS) if QKV_FP8 else SCALE
                    nc.scalar.activation(
                        out=eT_sb[:, ji, :], in_=ps, func=AF.Exp,
                        bias=nln16_sb, scale=float(escale),
                    )
                    # denominator for double-chunk jj interleaves two score
                    # groups later, when its exp results are already drained
                    if ji >= 3 and ji % 2 == 1:
                        denom_mm((ji - 3) // 2)
                denom_mm(NT // 2 - 1)
                recip = avpool.tile([128, N], F32, name="recip")
                nc.vector.reciprocal_approx_fast(out=recip, in_=ps_d)

                # av = (vT.T @ eT) * recip (bf16), fp8 DoubleRow over j
                av_sb = avpool.tile([128, KT, N], FP8 if PROJ_FP8 else BF16, name="av_sb")
                for ci in range(KT):
                    ps = ps_big.tile([128, N], F32, name="mmps")
                    for jj in range(NT // 2):
                        for ni in range(2):
                            nc.tensor.matmul(
                                ps[:, ni * 512 : (ni + 1) * 512],
                                lhsT=vT_sb[:, 2 * jj : 2 * jj + 2, ci * 128 : (ci + 1) * 128],
                                rhs=eT_sb[:, 2 * jj : 2 * jj + 2, ni * 512 : (ni + 1) * 512],
                                start=(jj == 0), stop=(jj == NT // 2 - 1),
                                perf_mode=mybir.MatmulPerfMode.DoubleRow,
                            )
                    for hf in range(2):
                        sl = slice(hf * 512, (hf + 1) * 512)
                        nc.vector.tensor_tensor(av_sb[:, ci, sl], ps[:, sl], recip[:, sl], OP.mult)

                # out = x + wprojT.T @ av + b_eff, single fused drain per tile
                for oi in range(KT):
                    ps = ps_big.tile([128, N], F32, name="mmps")
                    w_sl = wprojT[:, :, oi * 128 : (oi + 1) * 128]
                    if PROJ_FP8:
                        for kk in range(2):
                            for ni in range(2):
                                nc.tensor.matmul(
                                    ps[:, ni * 512 : (ni + 1) * 512],
                                    lhsT=w_sl[:, 2 * kk : 2 * kk + 2, :],
                                    rhs=av_sb[:, 2 * kk : 2 * kk + 2, ni * 512 : (ni + 1) * 512],
                                    start=(kk == 0), stop=(kk == 1),
                                    perf_mode=mybir.MatmulPerfMode.DoubleRow,
                                )
                    else:
                        for ci in range(KT):
                            for ni in range(2):
                                nc.tensor.matmul(
                                    ps[:, ni * 512 : (ni + 1) * 512],
                                    lhsT=w_sl[:, ci, :],
                                    rhs=av_sb[:, ci, ni * 512 : (ni + 1) * 512],
                                    start=(ci == 0), stop=(ci == KT - 1),
                                )
                    tmp = opool.tile([128, N], F32, name="tmp")
                    o_sb = opool.tile([128, N], F32, name="o_sb")
                    o_ext_sl = out_ext[b].rearrange("(ko p) n -> p ko n", p=128)[:, oi, :]
                    for hf in range(2):
                        sl = slice(hf * 512, (hf + 1) * 512)
                        if PROJ_FP8:
                            nc.vector.tensor_scalar(
                                out=tmp[:, sl], in0=ps[:, sl],
                                scalar1=1.0 / WS, scalar2=consts[:, 16 + oi : 17 + oi],
                                op0=OP.mult, op1=OP.add,
                            )
                        else:
                            nc.vector.tensor_scalar_add(
                                out=tmp[:, sl], in0=ps[:, sl],
                                scalar1=consts[:, 16 + oi : 17 + oi],
                            )
                        nc.gpsimd.tensor_tensor(
                            o_sb[:, sl], tmp[:, sl], x_sbs[b][:, oi, sl], OP.add
                        )
                        nc.gpsimd.dma_start(out=o_ext_sl[:, sl], in_=o_sb[:, sl])

            qks = [
                (
                    qkpool.tile([128, KT, N], FP8, name="q_sb"),
                    qkpool.tile([128, KT, N], FP8, name="k_sb"),
                )
                for _ in range(BPC)
            ]
            vTs = [vepool.tile([128, NT, C], FP8, name="vT_sb") for _ in range(BPC)]

            emit_gn(0)
            emit_qk(0, qks[0])
            emit_vt(0, vTs[0])
            emit_gn(1)          # batch 1 stats overlap batch 0's attention
            emit_attn(0, qks[0], vTs[0])
            emit_qk(1, qks[1])
            emit_vt(1, vTs[1])
            emit_attn(1, qks[1], vTs[1])

    nc.compile()
    return nc


_NC_CACHE = None


def _get_nc():
    global _NC_CACHE
    if _NC_CACHE is None:
        _NC_CACHE = build_nc()
    return _NC_CACHE


def _prep_consts(gamma, beta, w_qkv, b_qkv, w_proj, b_proj):
    bf = ml_dtypes.bfloat16
    f8 = ml_dtypes.float8_e4m3
    if QKV_FP8:
        wqkvT = np.ascontiguousarray(w_qkv.T * WS).astype(f8)  # [C, 3C]
    else:
        wqkvT = np.ascontiguousarray(w_qkv.T).astype(bf)
    if PROJ_FP8:
        wprojT = np.ascontiguousarray(w_proj.T * WS).astype(f8)  # [C, C]
    else:
        wprojT = np.ascontiguousarray(w_proj.T).astype(bf)
    b_q, b_k, b_v = b_qkv[0:C], b_qkv[C : 2 * C], b_qkv[2 * C : 3 * C]
    bias_s = WS if QKV_FP8 else 1.0
    b_eff = w_proj.astype(np.float64) @ b_v.astype(np.float64) + b_proj
    consts = np.stack(
        [gamma, beta, bias_s * b_q, bias_s * b_k, b_eff.astype(np.float32)], axis=0
    )  # [5, 512]
    consts = consts.reshape(5, 4, 128).transpose(2, 0, 1).reshape(128, 20)
    consts = np.ascontiguousarray(consts, dtype=np.float32)
    gmat = (np.kron(np.eye(8, dtype=np.float32), np.ones((16, 16), np.float32)) / 16.0).astype(bf)
    # denominator lhsT: value WS compensates vT carrying a factor of WS
    ones = np.full((128, 256), WS if QKV_FP8 else 1.0, f8)
    return wqkvT, wprojT, consts, gmat, ones


def make_in_maps(x, gamma, beta, w_qkv, b_qkv, w_proj, b_proj):
    x = np.asarray(x, np.float32)
    gamma = np.asarray(gamma, np.float32)
    beta = np.asarray(beta, np.float32)
    w_qkv = np.asarray(w_qkv, np.float32)
    b_qkv = np.asarray(b_qkv, np.float32)
    w_proj = np.asarray(w_proj, np.float32)
    b_proj = np.asarray(b_proj, np.float32)
    wqkvT, wprojT, consts, gmat, ones = _prep_consts(
        gamma, beta, w_qkv, b_qkv, w_proj, b_proj
    )
    xr = np.ascontiguousarray(x.reshape(B, C, N))
    return [
        {
            "x": xr[i * BPC : (i + 1) * BPC],
            "wqkvT": wqkvT,
            "wprojT": wprojT,
            "consts": consts,
            "gmat": gmat,
            "ones": ones,
        }
        for i in range(N_CORES)
    ]


def kernel(x, gamma, beta, w_qkv, b_qkv, w_proj, b_proj):
    from concourse.bass_utils import run_bass_kernel_spmd

    nc = _get_nc()
    in_maps = make_in_maps(x, gamma, beta, w_qkv, b_qkv, w_proj, b_proj)
    res = run_bass_kernel_spmd(nc, in_maps, core_ids=list(range(N_CORES)))
    out = np.concatenate([res.results[i]["out"] for i in range(N_CORES)], axis=0)
    return np.ascontiguousarray(out.reshape(B, C, H, W), dtype=np.float32)



# revision 2
# speedup vs baseline: 1.2388x; 1.2388x over previous
"""AttentionBlock (GroupNorm + single-head attention + proj + residual) on 8 TRN2
NeuronCores.

Reference computation (B=16, C=512, H=W=32, N=H*W=1024, 32 groups):
    h   = group_norm(x, gamma, beta)                      # [B,C,H,W]
    qkv = conv1x1(h, w_qkv) + b_qkv                       # [B,3C,H,W]
    s   = q^T k / sqrt(C); a = softmax(s, axis=-1)        # [B,N,N]
    o   = v @ a^T; out = x + conv1x1(o, w_proj) + b_proj  # [B,C,H,W]

Sharding: pure data-parallel over batch. B=16 -> 2 batch elements per core,
weights replicated, no collectives.

v2 structure (per batch element, all [partition, free]):
    x         : [c, n] bf16, 4 tiles of [128, 1024] (halves the input DMA)
    h         : [c, n] fp8, per-chunk GroupNorm pipelined behind the x DMA
    g = M h   : [c, n] fp8 where M = (Wq^T Wk)*WS is host-precomputed; this
                replaces BOTH the q and k projections (s = h'Mh). The k-bias
                term is a per-i additive constant in scores and cancels in the
                softmax; the q-bias term is zero for this model (asserted on
                host, with a numpy fallback path otherwise).
    vT        : [n, c] fp8, 8 x [128, 512] via swapped matmul
    eT=exp(.) : [j, i] fp8, 8 x [128, 1024] (softmax dim on partitions)
    denom     : ones-matmul over j -> reciprocal (applied post-AV; division
                commutes with the linear map)
    av        : [c, i] fp8 = vT^T @ eT, scaled by recip
    out       : PSUM accumulates w_projT^T @ av + WS*I @ x_bf16 (the residual
                rides the matmul); single DVE drain (x 1/WS, + b_eff) -> DMA.
b_eff = w_proj @ b_v + b_proj is folded on host.

All big matmuls are fp8 DoubleRow (weights pre-scaled by WS=8); PSUM is f32.
GroupNorm statistics are computed in f32 from the bf16 x.

Emission interleaves the two batch elements so the in-order engine streams
keep TensorE fed: b1's GN stats hide under b0's g/vt phases, b1's g/vt
matmuls fill the exp/recip latency inside b0's attention.
"""

import sys

for _p in ("/opt/trn_rl_repo", "/opt/pypackages"):
    if _p not in sys.path:
        sys.path.append(_p)

import numpy as np
import ml_dtypes

import concourse.bass as bass
import concourse.bacc as bacc
import concourse.tile as tile
from concourse import mybir

AF = mybir.ActivationFunctionType
OP = mybir.AluOpType
F32 = mybir.dt.float32
BF16 = mybir.dt.bfloat16
FP8 = mybir.dt.float8e4
DR = mybir.MatmulPerfMode.DoubleRow
LN16 = 2.772588722239781  # eT is stored as exp(s)/16 in fp8e4 to dodge the
                          # 448 saturation point; the softmax ratio is unchanged

N_CORES = 8
B, C, H, W = 16, 512, 32, 32
N = H * W               # 1024 pixels
BPC = B // N_CORES      # batch elements per core = 2
GROUPS = 32
EPS = 1e-5
KT = C // 128           # 4 contraction chunks over channels
NT = N // 128           # 8 chunks over pixels
SCALE = 1.0 / np.sqrt(np.float32(C))
WS = 8.0          # fp8 weight pre-scale (keeps N(0,1/512) weights out of subnormals)


def build_nc():
    nc = bacc.Bacc("TRN2", target_bir_lowering=False)

    x_ext = nc.declare_dram_parameter("x", [BPC, C, N], BF16, isOutput=False)
    wm_ext = nc.declare_dram_parameter("wm", [C, C], FP8, isOutput=False)
    wv_ext = nc.declare_dram_parameter("wv", [C, C], FP8, isOutput=False)
    wp_ext = nc.declare_dram_parameter("wp", [C, C], FP8, isOutput=False)
    # consts: [128, 12] f32 = gamma | beta | b_eff, each [128, 4]
    consts_ext = nc.declare_dram_parameter("consts", [128, 12], F32, isOutput=False)
    # gmat: 16x16 block-diagonal of 1/16 (group-mean matrix)
    gmat_ext = nc.declare_dram_parameter("gmat", [128, 128], BF16, isOutput=False)
    identw_ext = nc.declare_dram_parameter("identw", [128, 128], BF16, isOutput=False)
    ones_ext = nc.declare_dram_parameter("ones", [128, 256], FP8, isOutput=False)
    out_ext = nc.declare_dram_parameter("out", [BPC, C, N], F32, isOutput=True)

    with tile.TileContext(nc) as tc:
        with (
            tc.tile_pool(name="wpool", bufs=1) as wpool,
            tc.tile_pool(name="xpool", bufs=2) as xpool,
            tc.tile_pool(name="hpool", bufs=2) as hpool,
            tc.tile_pool(name="gpool", bufs=1) as gpool,
            tc.tile_pool(name="vepool", bufs=1) as vepool,
            tc.tile_pool(name="avpool", bufs=1) as avpool,
            tc.tile_pool(name="opool", bufs=2) as opool,
            tc.tile_pool(name="stpool", bufs=2) as stpool,
            tc.tile_pool(name="ps_big", bufs=3, space="PSUM") as ps_big,
            tc.tile_pool(name="ps_gn", bufs=1, space="PSUM") as ps_gn,
        ):
            # small constants first (gmat gates the first matmul)
            consts = wpool.tile([128, 12], F32)
            nc.scalar.dma_start(out=consts, in_=consts_ext[:])
            gmat = wpool.tile([128, 128], BF16)
            nc.scalar.dma_start(out=gmat, in_=gmat_ext[:])
            identw = wpool.tile([128, 128], BF16)
            nc.scalar.dma_start(out=identw, in_=identw_ext[:])
            ones = wpool.tile([128, 256], FP8)
            nc.scalar.dma_start(out=ones, in_=ones_ext[:])
            eps_sb = wpool.tile([128, 1], F32)
            nc.vector.memset(eps_sb, EPS)
            nln16_sb = wpool.tile([128, 1], F32)
            nc.vector.memset(nln16_sb, -LN16)
            gamma_sb = consts[:, 0:4]
            beta_sb = consts[:, 4:8]

            # x chunks stream on the sync queue in first-use order (b0 then
            # b1); weights stream on the scalar queue in parallel.
            x_sbs = []
            for b in range(BPC):
                x_sb = xpool.tile([128, KT, N], BF16, name="x_sb")
                x_sbs.append(x_sb)
            xr = [x_ext[b].rearrange("(ko p) n -> p ko n", p=128) for b in range(BPC)]
            for b in range(BPC):
                for ki in range(KT):
                    nc.sync.dma_start(out=x_sbs[b][:, ki, :], in_=xr[b][:, ki, :])

            wm = wpool.tile([128, KT, C], FP8)
            nc.scalar.dma_start(out=wm, in_=wm_ext[:].rearrange("(ko p) f -> p ko f", p=128))
            wv = wpool.tile([128, KT, C], FP8)
            nc.scalar.dma_start(out=wv, in_=wv_ext[:].rearrange("(ko p) f -> p ko f", p=128))
            wp = wpool.tile([128, KT, C], FP8)
            nc.scalar.dma_start(out=wp, in_=wp_ext[:].rearrange("(ko p) f -> p ko f", p=128))

            h_sbs = [
                hpool.tile([128, KT, N], FP8, name="h_sb") for _ in range(BPC)
            ]
            g_sbs = [gpool.tile([128, KT, N], FP8, name="g_sb") for _ in range(BPC)]
            vTs = [vepool.tile([128, NT, C], FP8, name="vT_sb") for _ in range(BPC)]

            def emit_gn_pair(b, pr):
                # GroupNorm for chunk pair (2*pr, 2*pr+1): per-partition
                # mean/var via bn_stats, group-reduce the 16-partition blocks
                # with one small matmul against gmat, then h = x*s + t (fp8).
                # Per-pair so h[0:2] is ready before x chunks 2,3 even land.
                x_sb = x_sbs[b]
                ks = (2 * pr, 2 * pr + 1)
                mv = stpool.tile([128, 2, 2], F32, name="mv")
                for j, ki in enumerate(ks):
                    stats = stpool.tile([128, 2, 6], F32, name="stats")
                    nc.vector.bn_stats(out=stats[:, 0, :], in_=x_sb[:, ki, 0:512])
                    nc.vector.bn_stats(out=stats[:, 1, :], in_=x_sb[:, ki, 512:1024])
                    nc.vector.bn_aggr(out=mv[:, j, :], in_=stats)
                msq = stpool.tile([128, 2], F32, name="msq")
                nc.vector.tensor_tensor(msq, mv[:, :, 0], mv[:, :, 0], OP.mult)
                nc.vector.tensor_tensor(mv[:, :, 1], mv[:, :, 1], msq, OP.add)
                mv_bf = stpool.tile([128, 4], BF16, name="mv_bf")
                nc.vector.tensor_copy(out=mv_bf, in_=mv.rearrange("p a b -> p (a b)"))
                gstat = ps_gn.tile([128, 128], F32, name="gstat", tag="gnps", bufs=1)[:, :4]
                nc.tensor.matmul(gstat, lhsT=gmat, rhs=mv_bf, start=True, stop=True)
                gs_sb = stpool.tile([128, 4], F32, name="gs_sb")
                nc.vector.tensor_copy(out=gs_sb, in_=gstat)
                gmean = gs_sb[:, 0:4:2]
                gex2 = gs_sb[:, 1:4:2]
                gmsq = stpool.tile([128, 2], F32, name="gmsq")
                nc.vector.tensor_tensor(gmsq, gmean, gmean, OP.mult)
                gvar = stpool.tile([128, 2], F32, name="gvar")
                nc.vector.tensor_tensor(gvar, gex2, gmsq, OP.subtract)
                gstd = stpool.tile([128, 2], F32, name="gstd")
                nc.scalar.activation(out=gstd, in_=gvar, func=AF.Sqrt, bias=eps_sb)
                rstd = stpool.tile([128, 2], F32, name="rstd")
                nc.vector.reciprocal(out=rstd, in_=gstd)
                scl = stpool.tile([128, 2], F32, name="scl")
                nc.vector.tensor_tensor(scl, rstd, gamma_sb[:, 2 * pr : 2 * pr + 2], OP.mult)
                mscl = stpool.tile([128, 2], F32, name="mscl")
                nc.vector.tensor_tensor(mscl, gmean, scl, OP.mult)
                sft = stpool.tile([128, 2], F32, name="sft")
                nc.vector.tensor_tensor(sft, beta_sb[:, 2 * pr : 2 * pr + 2], mscl, OP.subtract)
                h_sb = h_sbs[b]
                for j, ki in enumerate(ks):
                    if j == 0:
                        nc.vector.tensor_scalar(
                            out=h_sb[:, ki, :], in0=x_sb[:, ki, :],
                            scalar1=scl[:, j : j + 1], scalar2=sft[:, j : j + 1],
                            op0=OP.mult, op1=OP.add,
                        )
                    else:
                        nc.scalar.activation(
                            out=h_sb[:, ki, :], in_=x_sb[:, ki, :], func=AF.Identity,
                            bias=sft[:, j : j + 1], scale=scl[:, j : j + 1],
                        )

            def emit_g(b):
                # g = (WS*M)^T'... g[a, j] = sum_b M[a,b] h[b,j]; ACT drain
                h_sb = h_sbs[b]
                g_sb = g_sbs[b]
                for oi in range(KT):
                    ps = ps_big.tile([128, N], F32, name="mmps")
                    w_sl = wm[:, :, oi * 128 : (oi + 1) * 128]
                    for kk in range(2):
                        for ni in range(2):
                            nc.tensor.matmul(
                                ps[:, ni * 512 : (ni + 1) * 512],
                                lhsT=w_sl[:, 2 * kk : 2 * kk + 2, :],
                                rhs=h_sb[:, 2 * kk : 2 * kk + 2, ni * 512 : (ni + 1) * 512],
                                start=(kk == 0), stop=(kk == 1),
                                perf_mode=DR,
                            )
                    nc.scalar.activation(out=g_sb[:, oi, :], in_=ps, func=AF.Identity)

            def emit_vt(b):
                # vT = h.T @ wv (fp8 x WS), ACT Identity drain
                h_sb = h_sbs[b]
                vT_sb = vTs[b]
                for nn in range(NT // 2):
                    ps = ps_big.tile([128, N], F32, name="mmps")
                    for sub in range(2):
                        ni = 2 * nn + sub
                        for kk in range(2):
                            nc.tensor.matmul(
                                ps[:, sub * 512 : (sub + 1) * 512],
                                lhsT=h_sb[:, 2 * kk : 2 * kk + 2, ni * 128 : (ni + 1) * 128],
                                rhs=wv[:, 2 * kk : 2 * kk + 2, :],
                                start=(kk == 0), stop=(kk == 1),
                                perf_mode=DR,
                            )
                    nc.scalar.activation(
                        out=vT_sb[:, 2 * nn : 2 * nn + 2, :].rearrange("p a b -> p (a b)"),
                        in_=ps, func=AF.Identity,
                    )

            eT_sbs = [None, None]
            recips = [None, None]

            def emit_scores(b):
                # eT = exp(h.T g * SCALE/WS - ln16)  [j, i]; denominator
                # matmuls interleave behind the score matmuls; recip on DVE.
                h_sb = h_sbs[b]
                g_sb = g_sbs[b]
                eT_sb = vepool.tile([128, NT, N], FP8, name="eT_sb")
                eT_sbs[b] = eT_sb
                ps_d = ps_big.tile([128, N], F32, name="psden", tag="mmps")

                def denom_mm(jj):
                    for ni in range(2):
                        nc.tensor.matmul(
                            ps_d[:, ni * 512 : (ni + 1) * 512],
                            lhsT=ones.rearrange("p (two f) -> p two f", two=2),
                            rhs=eT_sb[:, 2 * jj : 2 * jj + 2, ni * 512 : (ni + 1) * 512],
                            start=(jj == 0), stop=(jj == NT // 2 - 1),
                            perf_mode=DR,
                        )

                for ji in range(NT):
                    ps = ps_big.tile([128, N], F32, name="mmps")
                    for kk in range(2):
                        for ni in range(2):
                            nc.tensor.matmul(
                                ps[:, ni * 512 : (ni + 1) * 512],
                                lhsT=g_sb[:, 2 * kk : 2 * kk + 2, ji * 128 : (ji + 1) * 128],
                                rhs=h_sb[:, 2 * kk : 2 * kk + 2, ni * 512 : (ni + 1) * 512],
                                start=(kk == 0), stop=(kk == 1),
                                perf_mode=DR,
                            )
                    nc.scalar.activation(
                        out=eT_sb[:, ji, :], in_=ps, func=AF.Exp,
                        bias=nln16_sb, scale=float(SCALE / WS),
                    )
                    # denominator for double-chunk jj interleaves two score
                    # groups later, when its exp results are already drained
                    if ji >= 3 and ji % 2 == 1:
                        denom_mm((ji - 3) // 2)
                denom_mm(NT // 2 - 1)
                recip = avpool.tile([128, N], F32, name="recip")
                recips[b] = recip
                nc.vector.reciprocal_approx_fast(out=recip, in_=ps_d)

            def emit_av(b):
                # av = (vT.T @ eT) * recip (fp8), DoubleRow over j
                vT_sb = vTs[b]
                eT_sb = eT_sbs[b]
                recip = recips[b]
                av_sb = avpool.tile([128, KT, N], FP8, name="av_sb")
                for ci in range(KT):
                    ps = ps_big.tile([128, N], F32, name="mmps")
                    for jj in range(NT // 2):
                        for ni in range(2):
                            nc.tensor.matmul(
                                ps[:, ni * 512 : (ni + 1) * 512],
                                lhsT=vT_sb[:, 2 * jj : 2 * jj + 2, ci * 128 : (ci + 1) * 128],
                                rhs=eT_sb[:, 2 * jj : 2 * jj + 2, ni * 512 : (ni + 1) * 512],
                                start=(jj == 0), stop=(jj == NT // 2 - 1),
                                perf_mode=DR,
                            )
                    nc.vector.tensor_tensor(av_sb[:, ci, :], ps, recip, OP.mult)
                return av_sb

            def emit_proj(b, av_sb):
                # out = x + wp.T @ av + b_eff; the residual is accumulated in
                # PSUM via a WS*I bf16 matmul on x, so the drain is one DVE op.
                x_sb = x_sbs[b]
                for oi in range(KT):
                    ps = ps_big.tile([128, N], F32, name="mmps")
                    w_sl = wp[:, :, oi * 128 : (oi + 1) * 128]
                    for kk in range(2):
                        for ni in range(2):
                            nc.tensor.matmul(
                                ps[:, ni * 512 : (ni + 1) * 512],
                                lhsT=w_sl[:, 2 * kk : 2 * kk + 2, :],
                                rhs=av_sb[:, 2 * kk : 2 * kk + 2, ni * 512 : (ni + 1) * 512],
                                start=(kk == 0), stop=False,
                                perf_mode=DR,
                            )
                    for ni in range(2):
                        nc.tensor.matmul(
                            ps[:, ni * 512 : (ni + 1) * 512],
                            lhsT=identw,
                            rhs=x_sb[:, oi, ni * 512 : (ni + 1) * 512],
                            start=False, stop=True,
                        )
                    o_sb = opool.tile([128, N], F32, name="o_sb")
                    nc.vector.tensor_scalar(
                        out=o_sb, in0=ps,
                        scalar1=1.0 / WS, scalar2=consts[:, 8 + oi : 9 + oi],
                        op0=OP.mult, op1=OP.add,
                    )
                    o_ext_sl = out_ext[b].rearrange("(ko p) n -> p ko n", p=128)[:, oi, :]
                    nc.gpsimd.dma_start(out=o_ext_sl, in_=o_sb)

            emit_gn_pair(0, 0)
            emit_gn_pair(0, 1)
            emit_g(0)
            emit_gn_pair(1, 0)       # b1 stats overlap b0's matmul phases
            emit_vt(0)
            emit_gn_pair(1, 1)
            emit_scores(0)
            emit_g(1)                # fills the exp/recip latency of b0
            av0 = emit_av(0)
            emit_vt(1)
            emit_proj(0, av0)
            emit_scores(1)
            av1 = emit_av(1)
            emit_proj(1, av1)

    nc.compile()
    return nc


_NC_CACHE = None


def _get_nc():
    global _NC_CACHE
    if _NC_CACHE is None:
        _NC_CACHE = build_nc()
    return _NC_CACHE


def _prep_consts(gamma, beta, w_qkv, b_qkv, w_proj, b_proj):
    bf = ml_dtypes.bfloat16
    f8 = ml_dtypes.float8_e4m3
    w_q, w_k, w_v = w_qkv[0:C], w_qkv[C : 2 * C], w_qkv[2 * C : 3 * C]
    b_v = b_qkv[2 * C : 3 * C]
    m = w_q.astype(np.float64).T @ w_k.astype(np.float64)  # [C, C]
    wm = np.ascontiguousarray(m.T * WS).astype(f8)         # lhsT layout [b, a]
    wv = np.ascontiguousarray(w_v.T * WS).astype(f8)
    wp = np.ascontiguousarray(w_proj.T * WS).astype(f8)
    b_eff = w_proj.astype(np.float64) @ b_v.astype(np.float64) + b_proj
    consts = np.stack([gamma, beta, b_eff.astype(np.float32)], axis=0)  # [3, 512]
    consts = consts.reshape(3, 4, 128).transpose(2, 0, 1).reshape(128, 12)
    consts = np.ascontiguousarray(consts, dtype=np.float32)
    gmat = (np.kron(np.eye(8, dtype=np.float32), np.ones((16, 16), np.float32)) / 16.0).astype(bf)
    identw = (WS * np.eye(128, dtype=np.float32)).astype(bf)
    # denominator lhsT: value WS compensates vT carrying a factor of WS
    ones = np.full((128, 256), WS, f8)
    return wm, wv, wp, consts, gmat, identw, ones


def make_in_maps(x, gamma, beta, w_qkv, b_qkv, w_proj, b_proj):
    bf = ml_dtypes.bfloat16
    x = np.asarray(x, np.float32)
    gamma = np.asarray(gamma, np.float32)
    beta = np.asarray(beta, np.float32)
    w_qkv = np.asarray(w_qkv, np.float32)
    b_qkv = np.asarray(b_qkv, np.float32)
    w_proj = np.asarray(w_proj, np.float32)
    b_proj = np.asarray(b_proj, np.float32)
    wm, wv, wp, consts, gmat, identw, ones = _prep_consts(
        gamma, beta, w_qkv, b_qkv, w_proj, b_proj
    )
    xr = np.ascontiguousarray(x.reshape(B, C, N).astype(bf))
    return [
        {
            "x": xr[i * BPC : (i + 1) * BPC],
            "wm": wm,
            "wv": wv,
            "wp": wp,
            "consts": consts,
            "gmat": gmat,
            "identw": identw,
            "ones": ones,
        }
        for i in range(N_CORES)
    ]


def _numpy_fallback(x, gamma, beta, w_qkv, b_qkv, w_proj, b_proj):
    # Exact reference implementation; only used when b_q is nonzero (the
    # device graph folds Wq^T Wk and drops the q-bias term, which is exact
    # for this model where b_qkv == 0).
    Bs, Cs, Hs, Ws_ = x.shape
    g = x.reshape(Bs, GROUPS, Cs // GROUPS, Hs, Ws_)
    mu = g.mean(axis=(2, 3, 4), keepdims=True)
    var = g.var(axis=(2, 3, 4), keepdims=True)
    g = (g - mu) / np.sqrt(var + EPS)
    h = g.reshape(Bs, Cs, Hs, Ws_) * gamma[None, :, None, None] + beta[None, :, None, None]
    hn = h.reshape(Bs, Cs, N)
    qkv = np.einsum("bcn,oc->bon", hn, w_qkv) + b_qkv[None, :, None]
    q, k, v = qkv[:, :Cs], qkv[:, Cs : 2 * Cs], qkv[:, 2 * Cs :]
    s = np.einsum("bci,bcj->bij", q, k) / np.sqrt(np.float32(Cs))
    s = s - s.max(axis=-1, keepdims=True)
    e = np.exp(s)
    a = e / e.sum(axis=-1, keepdims=True)
    o = np.einsum("bij,bcj->bci", a, v)
    o = np.einsum("bcn,oc->bon", o, w_proj) + b_proj[None, :, None]
    return (x + o.reshape(Bs, Cs, Hs, Ws_)).astype(np.float32)


def kernel(x, gamma, beta, w_qkv, b_qkv, w_proj, b_proj):
    from concourse.bass_utils import run_bass_kernel_spmd

    x = np.asarray(x, np.float32)
    b_qkv = np.asarray(b_qkv, np.float32)
    if np.abs(b_qkv[0:C]).max() > 1e-7:
        return _numpy_fallback(
            x, np.asarray(gamma, np.float32), np.asarray(beta, np.float32),
            np.asarray(w_qkv, np.float32), b_qkv,
            np.asarray(w_proj, np.float32), np.asarray(b_proj, np.float32),
        )

    nc = _get_nc()
    in_maps = make_in_maps(x, gamma, beta, w_qkv, b_qkv, w_proj, b_proj)
    res = run_bass_kernel_spmd(nc, in_maps, core_ids=list(range(N_CORES)))
    out = np.concatenate([res.results[i]["out"] for i in range(N_CORES)], axis=0)
    return np.ascontiguousarray(out.reshape(B, C, H, W), dtype=np.float32)


# revision 7
# speedup vs baseline: 1.2968x; 1.0468x over previous
"""AttentionBlock (GroupNorm + single-head attention + proj + residual) on 8 TRN2
NeuronCores.

Reference computation (B=16, C=512, H=W=32, N=H*W=1024, 32 groups):
    h   = group_norm(x, gamma, beta)                      # [B,C,H,W]
    qkv = conv1x1(h, w_qkv) + b_qkv                       # [B,3C,H,W]
    s   = q^T k / sqrt(C); a = softmax(s, axis=-1)        # [B,N,N]
    o   = v @ a^T; out = x + conv1x1(o, w_proj) + b_proj  # [B,C,H,W]

Sharding: pure data-parallel over batch. B=16 -> 2 batch elements per core,
weights replicated, no collectives.

v2 structure (per batch element, all [partition, free]):
    x         : [c, n] bf16, 4 tiles of [128, 1024] (halves the input DMA)
    h         : [c, n] fp8, per-chunk GroupNorm pipelined behind the x DMA
    g = M h   : [c, n] fp8 where M = (Wq^T Wk)*WS is host-precomputed; this
                replaces BOTH the q and k projections (s = h'Mh). The k-bias
                term is a per-i additive constant in scores and cancels in the
                softmax; the q-bias term is zero for this model (asserted on
                host, with a numpy fallback path otherwise).
    vT        : [n, c] fp8, 8 x [128, 512] via swapped matmul
    eT=exp(.) : [j, i] fp8, 8 x [128, 1024] (softmax dim on partitions)
    denom     : ones-matmul over j -> reciprocal (applied post-AV; division
                commutes with the linear map)
    av        : [c, i] fp8 = vT^T @ eT, scaled by recip
    out       : PSUM accumulates w_projT^T @ av + WS*I @ x_bf16 (the residual
                rides the matmul); single DVE drain (x 1/WS, + b_eff) -> DMA.
b_eff = w_proj @ b_v + b_proj is folded on host.

All big matmuls are fp8 DoubleRow (weights pre-scaled by WS=8); PSUM is f32.
GroupNorm statistics are computed in f32 from the bf16 x.

Emission interleaves the two batch elements so the in-order engine streams
keep TensorE fed: b1's GN stats hide under b0's g/vt phases, b1's g/vt
matmuls fill the exp/recip latency inside b0's attention.
"""

import sys

for _p in ("/opt/trn_rl_repo", "/opt/pypackages"):
    if _p not in sys.path:
        sys.path.append(_p)

import numpy as np
import ml_dtypes

import concourse.bass as bass
import concourse.bacc as bacc
import concourse.tile as tile
from concourse import mybir

AF = mybir.ActivationFunctionType
OP = mybir.AluOpType
F32 = mybir.dt.float32
BF16 = mybir.dt.bfloat16
FP8 = mybir.dt.float8e4
DR = mybir.MatmulPerfMode.DoubleRow
LN16 = 2.772588722239781  # eT is stored as exp(s)/16 in fp8e4 to dodge the
                          # 448 saturation point; the softmax ratio is unchanged

N_CORES = 8
B, C, H, W = 16, 512, 32, 32
N = H * W               # 1024 pixels
BPC = B // N_CORES      # batch elements per core = 2
GROUPS = 32
EPS = 1e-5
KT = C // 128           # 4 contraction chunks over channels
NT = N // 128           # 8 chunks over pixels
SCALE = 1.0 / np.sqrt(np.float32(C))
WS = 8.0          # fp8 weight pre-scale (keeps N(0,1/512) weights out of subnormals)


def build_nc():
    nc = bacc.Bacc("TRN2", target_bir_lowering=False)

    x_ext = nc.declare_dram_parameter("x", [BPC, C, N], BF16, isOutput=False)
    wm_ext = nc.declare_dram_parameter("wm", [C, C], FP8, isOutput=False)
    wv_ext = nc.declare_dram_parameter("wv", [C, C], FP8, isOutput=False)
    wp_ext = nc.declare_dram_parameter("wp", [C, C], FP8, isOutput=False)
    # consts: [128, 12] f32 = gamma | beta | b_eff, each [128, 4]
    consts_ext = nc.declare_dram_parameter("consts", [128, 12], F32, isOutput=False)
    # gmat: 16x16 block-diagonal of 1/16 (group-mean matrix)
    gmat_ext = nc.declare_dram_parameter("gmat", [128, 128], BF16, isOutput=False)
    identw_ext = nc.declare_dram_parameter("identw", [128, 128], BF16, isOutput=False)
    ones_ext = nc.declare_dram_parameter("ones", [128, 256], FP8, isOutput=False)
    out_ext = nc.declare_dram_parameter("out", [BPC, C, N], BF16, isOutput=True)

    with tile.TileContext(nc) as tc:
        with (
            tc.tile_pool(name="wpool", bufs=1) as wpool,
            tc.tile_pool(name="xpool", bufs=2) as xpool,
            tc.tile_pool(name="hpool", bufs=2) as hpool,
            tc.tile_pool(name="gpool", bufs=1) as gpool,
            tc.tile_pool(name="vepool", bufs=1) as vepool,
            tc.tile_pool(name="avpool", bufs=1) as avpool,
            tc.tile_pool(name="opool", bufs=2) as opool,
            tc.tile_pool(name="stpool", bufs=2) as stpool,
            tc.tile_pool(name="ps_big", bufs=3, space="PSUM") as ps_big,
            tc.tile_pool(name="ps_gn", bufs=1, space="PSUM") as ps_gn,
        ):
            # small constants first (gmat gates the first matmul)
            gmat = wpool.tile([128, 128], BF16)
            nc.scalar.dma_start(out=gmat, in_=gmat_ext[:])
            consts = wpool.tile([128, 12], F32)
            nc.scalar.dma_start(out=consts, in_=consts_ext[:])
            eps_sb = wpool.tile([128, 1], F32)
            nc.vector.memset(eps_sb, EPS)
            nln16_sb = wpool.tile([128, 1], F32)
            nc.vector.memset(nln16_sb, -LN16)
            gamma_sb = consts[:, 0:4]
            beta_sb = consts[:, 4:8]

            # x streams as chunk-PAIR DMAs (fewer descriptor-gen stalls),
            # b0 on the sync queue, b1 on the gpsimd queue; weights stream
            # on the scalar queue in parallel, in first-use order.
            x_sbs = []
            for b in range(BPC):
                x_sb = xpool.tile([128, KT, N], BF16, name="x_sb")
                x_sbs.append(x_sb)
            xr = [x_ext[b].rearrange("(ko p) n -> p ko n", p=128) for b in range(BPC)]
            for b, eng in ((0, nc.sync), (1, nc.gpsimd)):
                for pr in range(2):
                    eng.dma_start(
                        out=x_sbs[b][:, 2 * pr : 2 * pr + 2, :],
                        in_=xr[b][:, 2 * pr : 2 * pr + 2, :],
                    )

            wm = wpool.tile([128, KT, C], FP8)
            nc.scalar.dma_start(out=wm, in_=wm_ext[:].rearrange("(ko p) f -> p ko f", p=128))
            wv = wpool.tile([128, KT, C], FP8)
            nc.scalar.dma_start(out=wv, in_=wv_ext[:].rearrange("(ko p) f -> p ko f", p=128))
            identw = wpool.tile([128, 128], BF16)
            nc.scalar.dma_start(out=identw, in_=identw_ext[:])
            ones = wpool.tile([128, 256], FP8)
            nc.scalar.dma_start(out=ones, in_=ones_ext[:])
            wp = wpool.tile([128, KT, C], FP8)
            nc.scalar.dma_start(out=wp, in_=wp_ext[:].rearrange("(ko p) f -> p ko f", p=128))

            h_sbs = [
                hpool.tile([128, KT, N], FP8, name="h_sb") for _ in range(BPC)
            ]
            g_sbs = [gpool.tile([128, KT, N], FP8, name="g_sb") for _ in range(BPC)]
            vTs = [vepool.tile([128, NT, C], FP8, name="vT_sb") for _ in range(BPC)]

            def emit_gn_pair(b, pr):
                # GroupNorm for chunk pair (2*pr, 2*pr+1): per-partition
                # mean/var via bn_stats, group-reduce the 16-partition blocks
                # with one small matmul against gmat, then h = x*s + t (fp8).
                # Per-pair so h[0:2] is ready before x chunks 2,3 even land.
                x_sb = x_sbs[b]
                ks = (2 * pr, 2 * pr + 1)
                mv = stpool.tile([128, 2, 2], F32, name="mv")
                for j, ki in enumerate(ks):
                    stats = stpool.tile([128, 2, 6], F32, name="stats")
                    nc.vector.bn_stats(out=stats[:, 0, :], in_=x_sb[:, ki, 0:512])
                    nc.vector.bn_stats(out=stats[:, 1, :], in_=x_sb[:, ki, 512:1024])
                    nc.vector.bn_aggr(out=mv[:, j, :], in_=stats)
                msq = stpool.tile([128, 2], F32, name="msq")
                nc.vector.tensor_tensor(msq, mv[:, :, 0], mv[:, :, 0], OP.mult)
                nc.vector.tensor_tensor(mv[:, :, 1], mv[:, :, 1], msq, OP.add)
                mv_bf = stpool.tile([128, 4], BF16, name="mv_bf")
                nc.vector.tensor_copy(out=mv_bf, in_=mv.rearrange("p a b -> p (a b)"))
                gstat = ps_gn.tile([128, 128], F32, name="gstat", tag="gnps", bufs=1)[:, :4]
                nc.tensor.matmul(gstat, lhsT=gmat, rhs=mv_bf, start=True, stop=True)
                gs_sb = stpool.tile([128, 4], F32, name="gs_sb")
                nc.vector.tensor_copy(out=gs_sb, in_=gstat)
                gmean = gs_sb[:, 0:4:2]
                gex2 = gs_sb[:, 1:4:2]
                gmsq = stpool.tile([128, 2], F32, name="gmsq")
                nc.vector.tensor_tensor(gmsq, gmean, gmean, OP.mult)
                gvar = stpool.tile([128, 2], F32, name="gvar")
                nc.vector.tensor_tensor(gvar, gex2, gmsq, OP.subtract)
                gstd = stpool.tile([128, 2], F32, name="gstd")
                nc.scalar.activation(out=gstd, in_=gvar, func=AF.Sqrt, bias=eps_sb)
                rstd = stpool.tile([128, 2], F32, name="rstd")
                nc.vector.reciprocal(out=rstd, in_=gstd)
                scl = stpool.tile([128, 2], F32, name="scl")
                nc.vector.tensor_tensor(scl, rstd, gamma_sb[:, 2 * pr : 2 * pr + 2], OP.mult)
                mscl = stpool.tile([128, 2], F32, name="mscl")
                nc.vector.tensor_tensor(mscl, gmean, scl, OP.mult)
                sft = stpool.tile([128, 2], F32, name="sft")
                nc.vector.tensor_tensor(sft, beta_sb[:, 2 * pr : 2 * pr + 2], mscl, OP.subtract)
                # h drains on DVE only: ACT must stay clear for the exp
                # stream during the scores phases.
                h_sb = h_sbs[b]
                for j, ki in enumerate(ks):
                    nc.vector.tensor_scalar(
                        out=h_sb[:, ki, :], in0=x_sb[:, ki, :],
                        scalar1=scl[:, j : j + 1], scalar2=sft[:, j : j + 1],
                        op0=OP.mult, op1=OP.add,
                    )

            def emit_g(b):
                # g = (WS*M)^T'... g[a, j] = sum_b M[a,b] h[b,j]; ACT drain
                h_sb = h_sbs[b]
                g_sb = g_sbs[b]
                for oi in range(KT):
                    ps = ps_big.tile([128, N], F32, name="mmps")
                    w_sl = wm[:, :, oi * 128 : (oi + 1) * 128]
                    for kk in range(2):
                        for ni in range(2):
                            nc.tensor.matmul(
                                ps[:, ni * 512 : (ni + 1) * 512],
                                lhsT=w_sl[:, 2 * kk : 2 * kk + 2, :],
                                rhs=h_sb[:, 2 * kk : 2 * kk + 2, ni * 512 : (ni + 1) * 512],
                                start=(kk == 0), stop=(kk == 1),
                                perf_mode=DR,
                            )
                    nc.scalar.activation(out=g_sb[:, oi, :], in_=ps, func=AF.Identity)

            def emit_vt(b):
                # vT = h.T @ wv (fp8 x WS), ACT Identity drain
                h_sb = h_sbs[b]
                vT_sb = vTs[b]
                for nn in range(NT // 2):
                    ps = ps_big.tile([128, N], F32, name="mmps")
                    for sub in range(2):
                        ni = 2 * nn + sub
                        for kk in range(2):
                            nc.tensor.matmul(
                                ps[:, sub * 512 : (sub + 1) * 512],
                                lhsT=h_sb[:, 2 * kk : 2 * kk + 2, ni * 128 : (ni + 1) * 128],
                                rhs=wv[:, 2 * kk : 2 * kk + 2, :],
                                start=(kk == 0), stop=(kk == 1),
                                perf_mode=DR,
                            )
                    nc.scalar.activation(
                        out=vT_sb[:, 2 * nn : 2 * nn + 2, :].rearrange("p a b -> p (a b)"),
                        in_=ps, func=AF.Identity,
                    )

            eT_sbs = [None, None]
            recips = [None, None]

            def emit_scores(b):
                # eT = exp(h.T g * SCALE/WS - ln16)  [j, i]; denominator
                # matmuls interleave behind the score matmuls; recip on DVE.
                h_sb = h_sbs[b]
                g_sb = g_sbs[b]
                eT_sb = vepool.tile([128, NT, N], FP8, name="eT_sb")
                eT_sbs[b] = eT_sb
                ps_d = ps_big.tile([128, N], F32, name="psden", tag="mmps")

                def denom_mm(jj):
                    for ni in range(2):
                        nc.tensor.matmul(
                            ps_d[:, ni * 512 : (ni + 1) * 512],
                            lhsT=ones.rearrange("p (two f) -> p two f", two=2),
                            rhs=eT_sb[:, 2 * jj : 2 * jj + 2, ni * 512 : (ni + 1) * 512],
                            start=(jj == 0), stop=(jj == NT // 2 - 1),
                            perf_mode=DR,
                        )

                for ji in range(NT):
                    ps = ps_big.tile([128, N], F32, name="mmps")
                    for kk in range(2):
                        for ni in range(2):
                            nc.tensor.matmul(
                                ps[:, ni * 512 : (ni + 1) * 512],
                                lhsT=g_sb[:, 2 * kk : 2 * kk + 2, ji * 128 : (ji + 1) * 128],
                                rhs=h_sb[:, 2 * kk : 2 * kk + 2, ni * 512 : (ni + 1) * 512],
                                start=(kk == 0), stop=(kk == 1),
                                perf_mode=DR,
                            )
                    nc.scalar.activation(
                        out=eT_sb[:, ji, :], in_=ps, func=AF.Exp,
                        bias=nln16_sb, scale=float(SCALE / WS),
                    )
                    # denominator for double-chunk jj interleaves two score
                    # groups later, when its exp results are already drained
                    if ji >= 3 and ji % 2 == 1:
                        denom_mm((ji - 3) // 2)
                denom_mm(NT // 2 - 1)
                recip = avpool.tile([128, N], F32, name="recip")
                recips[b] = recip
                nc.vector.reciprocal_approx_fast(out=recip, in_=ps_d)

            def emit_av(b):
                # av = (vT.T @ eT) * recip (fp8), DoubleRow over j
                vT_sb = vTs[b]
                eT_sb = eT_sbs[b]
                recip = recips[b]
                av_sb = avpool.tile([128, KT, N], FP8, name="av_sb")
                for ci in range(KT):
                    ps = ps_big.tile([128, N], F32, name="mmps")
                    for jj in range(NT // 2):
                        for ni in range(2):
                            nc.tensor.matmul(
                                ps[:, ni * 512 : (ni + 1) * 512],
                                lhsT=vT_sb[:, 2 * jj : 2 * jj + 2, ci * 128 : (ci + 1) * 128],
                                rhs=eT_sb[:, 2 * jj : 2 * jj + 2, ni * 512 : (ni + 1) * 512],
                                start=(jj == 0), stop=(jj == NT // 2 - 1),
                                perf_mode=DR,
                            )
                    nc.vector.tensor_tensor(av_sb[:, ci, :], ps, recip, OP.mult)
                return av_sb

            def emit_proj(b, av_sb):
                # out = x + wp.T @ av + b_eff; the residual is accumulated in
                # PSUM via a WS*I bf16 matmul on x, so the drain is one DVE op.
                x_sb = x_sbs[b]
                for oi in range(KT):
                    ps = ps_big.tile([128, N], F32, name="mmps")
                    w_sl = wp[:, :, oi * 128 : (oi + 1) * 128]
                    for kk in range(2):
                        for ni in range(2):
                            nc.tensor.matmul(
                                ps[:, ni * 512 : (ni + 1) * 512],
                                lhsT=w_sl[:, 2 * kk : 2 * kk + 2, :],
                                rhs=av_sb[:, 2 * kk : 2 * kk + 2, ni * 512 : (ni + 1) * 512],
                                start=(kk == 0), stop=False,
                                perf_mode=DR,
                            )
                    for ni in range(2):
                        nc.tensor.matmul(
                            ps[:, ni * 512 : (ni + 1) * 512],
                            lhsT=identw,
                            rhs=x_sb[:, oi, ni * 512 : (ni + 1) * 512],
                            start=False, stop=True,
                        )
                    o_sb = opool.tile([128, N], BF16, name="o_sb")
                    nc.vector.tensor_scalar(
                        out=o_sb, in0=ps,
                        scalar1=1.0 / WS, scalar2=consts[:, 8 + oi : 9 + oi],
                        op0=OP.mult, op1=OP.add,
                    )
                    o_ext_sl = out_ext[b].rearrange("(ko p) n -> p ko n", p=128)[:, oi, :]
                    nc.gpsimd.dma_start(out=o_ext_sl, in_=o_sb)

            emit_gn_pair(0, 0)
            emit_gn_pair(0, 1)
            emit_g(0)
            emit_gn_pair(1, 0)       # b1 stats overlap b0's matmul phases
            emit_vt(0)
            emit_gn_pair(1, 1)
            emit_scores(0)
            emit_g(1)                # fills the exp/recip latency of b0
            av0 = emit_av(0)
            emit_vt(1)
            emit_scores(1)
            emit_proj(0, av0)        # fills the exp/recip latency of b1
            av1 = emit_av(1)
            emit_proj(1, av1)

    nc.compile()
    return nc


_NC_CACHE = None


def _get_nc():
    global _NC_CACHE
    if _NC_CACHE is None:
        _NC_CACHE = build_nc()
    return _NC_CACHE


def _prep_consts(gamma, beta, w_qkv, b_qkv, w_proj, b_proj):
    bf = ml_dtypes.bfloat16
    f8 = ml_dtypes.float8_e4m3
    w_q, w_k, w_v = w_qkv[0:C], w_qkv[C : 2 * C], w_qkv[2 * C : 3 * C]
    b_v = b_qkv[2 * C : 3 * C]
    m = w_q.astype(np.float64).T @ w_k.astype(np.float64)  # [C, C]
    wm = np.ascontiguousarray(m.T * WS).astype(f8)         # lhsT layout [b, a]
    wv = np.ascontiguousarray(w_v.T * WS).astype(f8)
    wp = np.ascontiguousarray(w_proj.T * WS).astype(f8)
    b_eff = w_proj.astype(np.float64) @ b_v.astype(np.float64) + b_proj
    consts = np.stack([gamma, beta, b_eff.astype(np.float32)], axis=0)  # [3, 512]
    consts = consts.reshape(3, 4, 128).transpose(2, 0, 1).reshape(128, 12)
    consts = np.ascontiguousarray(consts, dtype=np.float32)
    gmat = (np.kron(np.eye(8, dtype=np.float32), np.ones((16, 16), np.float32)) / 16.0).astype(bf)
    identw = (WS * np.eye(128, dtype=np.float32)).astype(bf)
    # denominator lhsT: value WS compensates vT carrying a factor of WS
    ones = np.full((128, 256), WS, f8)
    return wm, wv, wp, consts, gmat, identw, ones


def make_in_maps(x, gamma, beta, w_qkv, b_qkv, w_proj, b_proj):
    bf = ml_dtypes.bfloat16
    x = np.asarray(x, np.float32)
    gamma = np.asarray(gamma, np.float32)
    beta = np.asarray(beta, np.float32)
    w_qkv = np.asarray(w_qkv, np.float32)
    b_qkv = np.asarray(b_qkv, np.float32)
    w_proj = np.asarray(w_proj, np.float32)
    b_proj = np.asarray(b_proj, np.float32)
    wm, wv, wp, consts, gmat, identw, ones = _prep_consts(
        gamma, beta, w_qkv, b_qkv, w_proj, b_proj
    )
    xr = np.ascontiguousarray(x.reshape(B, C, N).astype(bf))
    return [
        {
            "x": xr[i * BPC : (i + 1) * BPC],
            "wm": wm,
            "wv": wv,
            "wp": wp,
            "consts": consts,
            "gmat": gmat,
            "identw": identw,
            "ones": ones,
        }
        for i in range(N_CORES)
    ]


def _numpy_fallback(x, gamma, beta, w_qkv, b_qkv, w_proj, b_proj):
    # Exact reference implementation; only used when b_q is nonzero (the
    # device graph folds Wq^T Wk and drops the q-bias term, which is exact
    # for this model where b_qkv == 0).
    Bs, Cs, Hs, Ws_ = x.shape
    g = x.reshape(Bs, GROUPS, Cs // GROUPS, Hs, Ws_)
    mu = g.mean(axis=(2, 3, 4), keepdims=True)
    var = g.var(axis=(2, 3, 4), keepdims=True)
    g = (g - mu) / np.sqrt(var + EPS)
    h = g.reshape(Bs, Cs, Hs, Ws_) * gamma[None, :, None, None] + beta[None, :, None, None]
    hn = h.reshape(Bs, Cs, N)
    qkv = np.einsum("bcn,oc->bon", hn, w_qkv) + b_qkv[None, :, None]
    q, k, v = qkv[:, :Cs], qkv[:, Cs : 2 * Cs], qkv[:, 2 * Cs :]
    s = np.einsum("bci,bcj->bij", q, k) / np.sqrt(np.float32(Cs))
    s = s - s.max(axis=-1, keepdims=True)
    e = np.exp(s)
    a = e / e.sum(axis=-1, keepdims=True)
    o = np.einsum("bij,bcj->bci", a, v)
    o = np.einsum("bcn,oc->bon", o, w_proj) + b_proj[None, :, None]
    return (x + o.reshape(Bs, Cs, Hs, Ws_)).astype(np.float32)


def kernel(x, gamma, beta, w_qkv, b_qkv, w_proj, b_proj):
    from concourse.bass_utils import run_bass_kernel_spmd

    x = np.asarray(x, np.float32)
    b_qkv = np.asarray(b_qkv, np.float32)
    if np.abs(b_qkv[0:C]).max() > 1e-7:
        return _numpy_fallback(
            x, np.asarray(gamma, np.float32), np.asarray(beta, np.float32),
            np.asarray(w_qkv, np.float32), b_qkv,
            np.asarray(w_proj, np.float32), np.asarray(b_proj, np.float32),
        )

    nc = _get_nc()
    in_maps = make_in_maps(x, gamma, beta, w_qkv, b_qkv, w_proj, b_proj)
    res = run_bass_kernel_spmd(nc, in_maps, core_ids=list(range(N_CORES)))
    out = np.concatenate([res.results[i]["out"] for i in range(N_CORES)], axis=0)
    return np.ascontiguousarray(out.reshape(B, C, H, W), dtype=np.float32)


# revision 9
# speedup vs baseline: 1.3268x; 1.0231x over previous
"""AttentionBlock (GroupNorm + single-head attention + proj + residual) on 8 TRN2
NeuronCores.

Reference computation (B=16, C=512, H=W=32, N=H*W=1024, 32 groups):
    h   = group_norm(x, gamma, beta)                      # [B,C,H,W]
    qkv = conv1x1(h, w_qkv) + b_qkv                       # [B,3C,H,W]
    s   = q^T k / sqrt(C); a = softmax(s, axis=-1)        # [B,N,N]
    o   = v @ a^T; out = x + conv1x1(o, w_proj) + b_proj  # [B,C,H,W]

Sharding: pure data-parallel over batch. B=16 -> 2 batch elements per core,
weights replicated, no collectives.

v2 structure (per batch element, all [partition, free]):
    x         : [c, n] bf16, 4 tiles of [128, 1024] (halves the input DMA)
    h         : [c, n] fp8, per-chunk GroupNorm pipelined behind the x DMA
    g = M h   : [c, n] fp8 where M = (Wq^T Wk)*WS is host-precomputed; this
                replaces BOTH the q and k projections (s = h'Mh). The k-bias
                term is a per-i additive constant in scores and cancels in the
                softmax; the q-bias term is zero for this model (asserted on
                host, with a numpy fallback path otherwise).
    vT        : [n, c] fp8, 8 x [128, 512] via swapped matmul
    eT=exp(.) : [j, i] fp8, 8 x [128, 1024] (softmax dim on partitions)
    denom     : ones-matmul over j -> reciprocal (applied post-AV; division
                commutes with the linear map)
    av        : [c, i] fp8 = vT^T @ eT, scaled by recip
    out       : PSUM accumulates w_projT^T @ av + WS*I @ x_bf16 (the residual
                rides the matmul); single DVE drain (x 1/WS, + b_eff) -> DMA.
b_eff = w_proj @ b_v + b_proj is folded on host.

All big matmuls are fp8 DoubleRow (weights pre-scaled by WS=8); PSUM is f32.
GroupNorm statistics are computed in f32 from the bf16 x.

Emission interleaves the two batch elements so the in-order engine streams
keep TensorE fed: b1's GN stats hide under b0's g/vt phases, b1's g/vt
matmuls fill the exp/recip latency inside b0's attention.
"""

import sys

for _p in ("/opt/trn_rl_repo", "/opt/pypackages"):
    if _p not in sys.path:
        sys.path.append(_p)

import numpy as np
import ml_dtypes

import concourse.bass as bass
import concourse.bacc as bacc
import concourse.tile as tile
from concourse import mybir

AF = mybir.ActivationFunctionType
OP = mybir.AluOpType
F32 = mybir.dt.float32
BF16 = mybir.dt.bfloat16
FP8 = mybir.dt.float8e4
DR = mybir.MatmulPerfMode.DoubleRow
LN16 = 2.772588722239781  # eT is stored as exp(s)/16 in fp8e4 to dodge the
                          # 448 saturation point; the softmax ratio is unchanged

N_CORES = 8
B, C, H, W = 16, 512, 32, 32
N = H * W               # 1024 pixels
BPC = B // N_CORES      # batch elements per core = 2
GROUPS = 32
EPS = 1e-5
KT = C // 128           # 4 contraction chunks over channels
NT = N // 128           # 8 chunks over pixels
SCALE = 1.0 / np.sqrt(np.float32(C))
WS = 8.0          # fp8 weight pre-scale (keeps N(0,1/512) weights out of subnormals)


def build_nc():
    nc = bacc.Bacc("TRN2", target_bir_lowering=False)

    x_ext = nc.declare_dram_parameter("x", [BPC, C, N], BF16, isOutput=False)
    wm_ext = nc.declare_dram_parameter("wm", [C, C], FP8, isOutput=False)
    wv_ext = nc.declare_dram_parameter("wv", [C, C], FP8, isOutput=False)
    wp_ext = nc.declare_dram_parameter("wp", [C, C], FP8, isOutput=False)
    # consts: [128, 12] f32 = gamma | beta | b_eff, each [128, 4]
    consts_ext = nc.declare_dram_parameter("consts", [128, 12], F32, isOutput=False)
    # gmat: 16x16 block-diagonal of 1/16 (group-mean matrix)
    gmat_ext = nc.declare_dram_parameter("gmat", [128, 128], BF16, isOutput=False)
    identw_ext = nc.declare_dram_parameter("identw", [128, 128], BF16, isOutput=False)
    ones_ext = nc.declare_dram_parameter("ones", [128, 256], FP8, isOutput=False)
    out_ext = nc.declare_dram_parameter("out", [BPC, C, N], BF16, isOutput=True)

    with tile.TileContext(nc) as tc:
        with (
            tc.tile_pool(name="wpool", bufs=1) as wpool,
            tc.tile_pool(name="xpool", bufs=2) as xpool,
            tc.tile_pool(name="hpool", bufs=2) as hpool,
            tc.tile_pool(name="gpool", bufs=1) as gpool,
            tc.tile_pool(name="vepool", bufs=1) as vepool,
            tc.tile_pool(name="avpool", bufs=1) as avpool,
            tc.tile_pool(name="opool", bufs=2) as opool,
            tc.tile_pool(name="stpool", bufs=2) as stpool,
            tc.tile_pool(name="ps_big", bufs=3, space="PSUM") as ps_big,
            tc.tile_pool(name="ps_gn", bufs=1, space="PSUM") as ps_gn,
        ):
            # ALL input DMAs are issued from the sync queue, in first-use
            # order: the scalar/ACT queue must stay clear for the exp stream
            # and DVE for stats. b0's first two chunks go as separate small
            # DMAs so GroupNorm stats can start at the earliest possible
            # moment; the rest ride as chunk-pairs.
            eps_sb = wpool.tile([128, 1], F32)
            nc.vector.memset(eps_sb, EPS)
            nln16_sb = wpool.tile([128, 1], F32)
            nc.vector.memset(nln16_sb, -LN16)

            gmat = wpool.tile([128, 128], BF16)
            nc.sync.dma_start(out=gmat, in_=gmat_ext[:])
            consts = wpool.tile([128, 12], F32)
            nc.sync.dma_start(out=consts, in_=consts_ext[:])
            gamma_sb = consts[:, 0:4]
            beta_sb = consts[:, 4:8]

            x_sbs = []
            for b in range(BPC):
                x_sb = xpool.tile([128, KT, N], BF16, name="x_sb")
                x_sbs.append(x_sb)
            xr = [x_ext[b].rearrange("(ko p) n -> p ko n", p=128) for b in range(BPC)]
            nc.sync.dma_start(out=x_sbs[0][:, 0, :], in_=xr[0][:, 0, :])
            nc.sync.dma_start(out=x_sbs[0][:, 1, :], in_=xr[0][:, 1, :])
            nc.sync.dma_start(out=x_sbs[0][:, 2:4, :], in_=xr[0][:, 2:4, :])
            wm = wpool.tile([128, KT, C], FP8)
            nc.sync.dma_start(out=wm, in_=wm_ext[:].rearrange("(ko p) f -> p ko f", p=128))
            nc.sync.dma_start(out=x_sbs[1][:, 0:2, :], in_=xr[1][:, 0:2, :])
            wv = wpool.tile([128, KT, C], FP8)
            nc.sync.dma_start(out=wv, in_=wv_ext[:].rearrange("(ko p) f -> p ko f", p=128))
            nc.sync.dma_start(out=x_sbs[1][:, 2:4, :], in_=xr[1][:, 2:4, :])
            ones = wpool.tile([128, 256], FP8)
            nc.sync.dma_start(out=ones, in_=ones_ext[:])
            identw = wpool.tile([128, 128], BF16)
            nc.sync.dma_start(out=identw, in_=identw_ext[:])
            wp = wpool.tile([128, KT, C], FP8)
            nc.sync.dma_start(out=wp, in_=wp_ext[:].rearrange("(ko p) f -> p ko f", p=128))

            h_sbs = [
                hpool.tile([128, KT, N], FP8, name="h_sb") for _ in range(BPC)
            ]
            g_sbs = [gpool.tile([128, KT, N], FP8, name="g_sb") for _ in range(BPC)]
            vTs = [vepool.tile([128, NT, C], FP8, name="vT_sb") for _ in range(BPC)]

            def emit_gn_pair(b, pr):
                # GroupNorm for chunk pair (2*pr, 2*pr+1): per-partition
                # mean/var via bn_stats, group-reduce the 16-partition blocks
                # with one small matmul against gmat, then h = x*s + t (fp8).
                # Per-pair so h[0:2] is ready before x chunks 2,3 even land.
                x_sb = x_sbs[b]
                ks = (2 * pr, 2 * pr + 1)
                mv = stpool.tile([128, 2, 2], F32, name="mv")
                for j, ki in enumerate(ks):
                    stats = stpool.tile([128, 2, 6], F32, name="stats")
                    nc.vector.bn_stats(out=stats[:, 0, :], in_=x_sb[:, ki, 0:512])
                    nc.vector.bn_stats(out=stats[:, 1, :], in_=x_sb[:, ki, 512:1024])
                    nc.vector.bn_aggr(out=mv[:, j, :], in_=stats)
                msq = stpool.tile([128, 2], F32, name="msq")
                nc.vector.tensor_tensor(msq, mv[:, :, 0], mv[:, :, 0], OP.mult)
                nc.vector.tensor_tensor(mv[:, :, 1], mv[:, :, 1], msq, OP.add)
                mv_bf = stpool.tile([128, 4], BF16, name="mv_bf")
                nc.vector.tensor_copy(out=mv_bf, in_=mv.rearrange("p a b -> p (a b)"))
                gstat = ps_gn.tile([128, 128], F32, name="gstat", tag="gnps", bufs=1)[:, :4]
                nc.tensor.matmul(gstat, lhsT=gmat, rhs=mv_bf, start=True, stop=True)
                gs_sb = stpool.tile([128, 4], F32, name="gs_sb")
                nc.vector.tensor_copy(out=gs_sb, in_=gstat)
                gmean = gs_sb[:, 0:4:2]
                gex2 = gs_sb[:, 1:4:2]
                gmsq = stpool.tile([128, 2], F32, name="gmsq")
                nc.vector.tensor_tensor(gmsq, gmean, gmean, OP.mult)
                gvar = stpool.tile([128, 2], F32, name="gvar")
                nc.vector.tensor_tensor(gvar, gex2, gmsq, OP.subtract)
                gstd = stpool.tile([128, 2], F32, name="gstd")
                nc.scalar.activation(out=gstd, in_=gvar, func=AF.Sqrt, bias=eps_sb)
                rstd = stpool.tile([128, 2], F32, name="rstd")
                nc.vector.reciprocal(out=rstd, in_=gstd)
                scl = stpool.tile([128, 2], F32, name="scl")
                nc.vector.tensor_tensor(scl, rstd, gamma_sb[:, 2 * pr : 2 * pr + 2], OP.mult)
                mscl = stpool.tile([128, 2], F32, name="mscl")
                nc.vector.tensor_tensor(mscl, gmean, scl, OP.mult)
                sft = stpool.tile([128, 2], F32, name="sft")
                nc.vector.tensor_tensor(sft, beta_sb[:, 2 * pr : 2 * pr + 2], mscl, OP.subtract)
                # h drains on DVE only: ACT must stay clear for the exp
                # stream during the scores phases.
                h_sb = h_sbs[b]
                for j, ki in enumerate(ks):
                    nc.vector.tensor_scalar(
                        out=h_sb[:, ki, :], in0=x_sb[:, ki, :],
                        scalar1=scl[:, j : j + 1], scalar2=sft[:, j : j + 1],
                        op0=OP.mult, op1=OP.add,
                    )

            def emit_g(b):
                # g = (WS*M)^T'... g[a, j] = sum_b M[a,b] h[b,j]; ACT drain
                h_sb = h_sbs[b]
                g_sb = g_sbs[b]
                for oi in range(KT):
                    ps = ps_big.tile([128, N], F32, name="mmps")
                    w_sl = wm[:, :, oi * 128 : (oi + 1) * 128]
                    for kk in range(2):
                        for ni in range(2):
                            nc.tensor.matmul(
                                ps[:, ni * 512 : (ni + 1) * 512],
                                lhsT=w_sl[:, 2 * kk : 2 * kk + 2, :],
                                rhs=h_sb[:, 2 * kk : 2 * kk + 2, ni * 512 : (ni + 1) * 512],
                                start=(kk == 0), stop=(kk == 1),
                                perf_mode=DR,
                            )
                    nc.scalar.activation(out=g_sb[:, oi, :], in_=ps, func=AF.Identity)

            def emit_vt(b):
                # vT = h.T @ wv (fp8 x WS), ACT Identity drain
                h_sb = h_sbs[b]
                vT_sb = vTs[b]
                for nn in range(NT // 2):
                    ps = ps_big.tile([128, N], F32, name="mmps")
                    for sub in range(2):
                        ni = 2 * nn + sub
                        for kk in range(2):
                            nc.tensor.matmul(
                                ps[:, sub * 512 : (sub + 1) * 512],
                                lhsT=h_sb[:, 2 * kk : 2 * kk + 2, ni * 128 : (ni + 1) * 128],
                                rhs=wv[:, 2 * kk : 2 * kk + 2, :],
                                start=(kk == 0), stop=(kk == 1),
                                perf_mode=DR,
                            )
                    nc.scalar.activation(
                        out=vT_sb[:, 2 * nn : 2 * nn + 2, :].rearrange("p a b -> p (a b)"),
                        in_=ps, func=AF.Identity,
                    )

            eT_sbs = [None, None]
            recips = [None, None]

            def emit_scores(b):
                # eT = exp(h.T g * SCALE/WS - ln16)  [j, i]; denominator
                # matmuls interleave behind the score matmuls; recip on DVE.
                h_sb = h_sbs[b]
                g_sb = g_sbs[b]
                eT_sb = vepool.tile([128, NT, N], FP8, name="eT_sb")
                eT_sbs[b] = eT_sb
                ps_d = ps_big.tile([128, N], F32, name="psden", tag="mmps")

                def denom_mm(jj):
                    for ni in range(2):
                        nc.tensor.matmul(
                            ps_d[:, ni * 512 : (ni + 1) * 512],
                            lhsT=ones.rearrange("p (two f) -> p two f", two=2),
                            rhs=eT_sb[:, 2 * jj : 2 * jj + 2, ni * 512 : (ni + 1) * 512],
                            start=(jj == 0), stop=(jj == NT // 2 - 1),
                            perf_mode=DR,
                        )

                for ji in range(NT):
                    ps = ps_big.tile([128, N], F32, name="mmps")
                    for kk in range(2):
                        for ni in range(2):
                            nc.tensor.matmul(
                                ps[:, ni * 512 : (ni + 1) * 512],
                                lhsT=g_sb[:, 2 * kk : 2 * kk + 2, ji * 128 : (ji + 1) * 128],
                                rhs=h_sb[:, 2 * kk : 2 * kk + 2, ni * 512 : (ni + 1) * 512],
                                start=(kk == 0), stop=(kk == 1),
                                perf_mode=DR,
                            )
                    nc.scalar.activation(
                        out=eT_sb[:, ji, :], in_=ps, func=AF.Exp,
                        bias=nln16_sb, scale=float(SCALE / WS),
                    )
                    # denominator for double-chunk jj interleaves two score
                    # groups later, when its exp results are already drained
                    if ji >= 3 and ji % 2 == 1:
                        denom_mm((ji - 3) // 2)
                denom_mm(NT // 2 - 1)
                recip = avpool.tile([128, N], F32, name="recip")
                recips[b] = recip
                nc.vector.reciprocal_approx_fast(out=recip, in_=ps_d)

            def emit_av(b):
                # av = (vT.T @ eT) * recip (fp8), DoubleRow over j
                vT_sb = vTs[b]
                eT_sb = eT_sbs[b]
                recip = recips[b]
                av_sb = avpool.tile([128, KT, N], FP8, name="av_sb")
                for ci in range(KT):
                    ps = ps_big.tile([128, N], F32, name="mmps")
                    for jj in range(NT // 2):
                        for ni in range(2):
                            nc.tensor.matmul(
                                ps[:, ni * 512 : (ni + 1) * 512],
                                lhsT=vT_sb[:, 2 * jj : 2 * jj + 2, ci * 128 : (ci + 1) * 128],
                                rhs=eT_sb[:, 2 * jj : 2 * jj + 2, ni * 512 : (ni + 1) * 512],
                                start=(jj == 0), stop=(jj == NT // 2 - 1),
                                perf_mode=DR,
                            )
                    nc.vector.tensor_tensor(av_sb[:, ci, :], ps, recip, OP.mult)
                return av_sb

            def emit_proj(b, av_sb):
                # out = x + wp.T @ av + b_eff; the residual is accumulated in
                # PSUM via a WS*I bf16 matmul on x, so the drain is one DVE op.
                x_sb = x_sbs[b]
                for oi in range(KT):
                    ps = ps_big.tile([128, N], F32, name="mmps")
                    w_sl = wp[:, :, oi * 128 : (oi + 1) * 128]
                    for kk in range(2):
                        for ni in range(2):
                            nc.tensor.matmul(
                                ps[:, ni * 512 : (ni + 1) * 512],
                                lhsT=w_sl[:, 2 * kk : 2 * kk + 2, :],
                                rhs=av_sb[:, 2 * kk : 2 * kk + 2, ni * 512 : (ni + 1) * 512],
                                start=(kk == 0), stop=False,
                                perf_mode=DR,
                            )
                    for ni in range(2):
                        nc.tensor.matmul(
                            ps[:, ni * 512 : (ni + 1) * 512],
                            lhsT=identw,
                            rhs=x_sb[:, oi, ni * 512 : (ni + 1) * 512],
                            start=False, stop=True,
                        )
                    # drains alternate DVE/ACT and the DMA issues alternate
                    # gpsimd/sync so the final tiles pipeline instead of
                    # serializing on one engine
                    o_sb = opool.tile([128, N], BF16, name="o_sb")
                    if oi % 2 == 0:
                        nc.vector.tensor_scalar(
                            out=o_sb, in0=ps,
                            scalar1=1.0 / WS, scalar2=consts[:, 8 + oi : 9 + oi],
                            op0=OP.mult, op1=OP.add,
                        )
                    else:
                        nc.scalar.activation(
                            out=o_sb, in_=ps, func=AF.Identity,
                            bias=consts[:, 8 + oi : 9 + oi], scale=1.0 / WS,
                        )
                    o_ext_sl = out_ext[b].rearrange("(ko p) n -> p ko n", p=128)[:, oi, :]
                    eng = nc.gpsimd if oi % 2 == 0 else nc.sync
                    eng.dma_start(out=o_ext_sl, in_=o_sb)

            emit_gn_pair(0, 0)
            emit_gn_pair(0, 1)
            emit_g(0)
            emit_gn_pair(1, 0)       # b1 stats overlap b0's matmul phases
            emit_vt(0)
            emit_gn_pair(1, 1)
            emit_scores(0)
            emit_g(1)                # fills the exp/recip latency of b0
            av0 = emit_av(0)
            emit_vt(1)
            emit_scores(1)
            emit_proj(0, av0)        # fills the exp/recip latency of b1
            av1 = emit_av(1)
            emit_proj(1, av1)

    nc.compile()
    return nc


_NC_CACHE = None


def _get_nc():
    global _NC_CACHE
    if _NC_CACHE is None:
        _NC_CACHE = build_nc()
    return _NC_CACHE


def _prep_consts(gamma, beta, w_qkv, b_qkv, w_proj, b_proj):
    bf = ml_dtypes.bfloat16
    f8 = ml_dtypes.float8_e4m3
    w_q, w_k, w_v = w_qkv[0:C], w_qkv[C : 2 * C], w_qkv[2 * C : 3 * C]
    b_v = b_qkv[2 * C : 3 * C]
    m = w_q.astype(np.float64).T @ w_k.astype(np.float64)  # [C, C]
    wm = np.ascontiguousarray(m.T * WS).astype(f8)         # lhsT layout [b, a]
    wv = np.ascontiguousarray(w_v.T * WS).astype(f8)
    wp = np.ascontiguousarray(w_proj.T * WS).astype(f8)
    b_eff = w_proj.astype(np.float64) @ b_v.astype(np.float64) + b_proj
    consts = np.stack([gamma, beta, b_eff.astype(np.float32)], axis=0)  # [3, 512]
    consts = consts.reshape(3, 4, 128).transpose(2, 0, 1).reshape(128, 12)
    consts = np.ascontiguousarray(consts, dtype=np.float32)
    gmat = (np.kron(np.eye(8, dtype=np.float32), np.ones((16, 16), np.float32)) / 16.0).astype(bf)
    identw = (WS * np.eye(128, dtype=np.float32)).astype(bf)
    # denominator lhsT: value WS compensates vT carrying a factor of WS
    ones = np.full((128, 256), WS, f8)
    return wm, wv, wp, consts, gmat, identw, ones


def make_in_maps(x, gamma, beta, w_qkv, b_qkv, w_proj, b_proj):
    bf = ml_dtypes.bfloat16
    x = np.asarray(x, np.float32)
    gamma = np.asarray(gamma, np.float32)
    beta = np.asarray(beta, np.float32)
    w_qkv = np.asarray(w_qkv, np.float32)
    b_qkv = np.asarray(b_qkv, np.float32)
    w_proj = np.asarray(w_proj, np.float32)
    b_proj = np.asarray(b_proj, np.float32)
    wm, wv, wp, consts, gmat, identw, ones = _prep_consts(
        gamma, beta, w_qkv, b_qkv, w_proj, b_proj
    )
    xr = np.ascontiguousarray(x.reshape(B, C, N).astype(bf))
    return [
        {
            "x": xr[i * BPC : (i + 1) * BPC],
            "wm": wm,
            "wv": wv,
            "wp": wp,
            "consts": consts,
            "gmat": gmat,
            "identw": identw,
            "ones": ones,
        }
        for i in range(N_CORES)
    ]


def _numpy_fallback(x, gamma, beta, w_qkv, b_qkv, w_proj, b_proj):
    # Exact reference implementation; only used when b_q is nonzero (the
    # device graph folds Wq^T Wk and drops the q-bias term, which is exact
    # for this model where b_qkv == 0).
    Bs, Cs, Hs, Ws_ = x.shape
    g = x.reshape(Bs, GROUPS, Cs // GROUPS, Hs, Ws_)
    mu = g.mean(axis=(2, 3, 4), keepdims=True)
    var = g.var(axis=(2, 3, 4), keepdims=True)
    g = (g - mu) / np.sqrt(var + EPS)
    h = g.reshape(Bs, Cs, Hs, Ws_) * gamma[None, :, None, None] + beta[None, :, None, None]
    hn = h.reshape(Bs, Cs, N)
    qkv = np.einsum("bcn,oc->bon", hn, w_qkv) + b_qkv[None, :, None]
    q, k, v = qkv[:, :Cs], qkv[:, Cs : 2 * Cs], qkv[:, 2 * Cs :]
    s = np.einsum("bci,bcj->bij", q, k) / np.sqrt(np.float32(Cs))
    s = s - s.max(axis=-1, keepdims=True)
    e = np.exp(s)
    a = e / e.sum(axis=-1, keepdims=True)
    o = np.einsum("bij,bcj->bci", a, v)
    o = np.einsum("bcn,oc->bon", o, w_proj) + b_proj[None, :, None]
    return (x + o.reshape(Bs, Cs, Hs, Ws_)).astype(np.float32)


def kernel(x, gamma, beta, w_qkv, b_qkv, w_proj, b_proj):
    from concourse.bass_utils import run_bass_kernel_spmd

    x = np.asarray(x, np.float32)
    b_qkv = np.asarray(b_qkv, np.float32)
    if np.abs(b_qkv[0:C]).max() > 1e-7:
        return _numpy_fallback(
            x, np.asarray(gamma, np.float32), np.asarray(beta, np.float32),
            np.asarray(w_qkv, np.float32), b_qkv,
            np.asarray(w_proj, np.float32), np.asarray(b_proj, np.float32),
        )

    nc = _get_nc()
    in_maps = make_in_maps(x, gamma, beta, w_qkv, b_qkv, w_proj, b_proj)
    res = run_bass_kernel_spmd(nc, in_maps, core_ids=list(range(N_CORES)))
    out = np.concatenate([res.results[i]["out"] for i in range(N_CORES)], axis=0)
    return np.ascontiguousarray(out.reshape(B, C, H, W), dtype=np.float32)
